# revision 48
# baseline (speedup 1.0000x reference)
# BitAttention (ternary-quantized GQA transformer block) on 8 Trainium2 NeuronCores.
#
# Reference computation (see problem):
#   w_q = sign(w) * mean(|w|)            (per weight tensor, global scale)
#   q = x @ w_q(wq).T ; k = x @ w_q(wk).T ; v = x @ w_q(wv).T
#   GQA causal attention (32 q heads, 8 kv heads, head_dim 64)
#   out = attn @ w_q(wo).T
#
# Sharding (8 cores): batch (2) x kv-head-group (4).  Each core computes
# attention for 2 kv heads / 8 q heads of one batch and a partial out-proj
# over its 512 attention-output features; the host sums 4 partials per batch.
#
# Device layout: activations are feature-major ("transposed", [feat, token]).
# Inputs enter pre-transposed/sliced in bf16; sign() is computed on device;
# the quant scales enter as a tiny [1,2] fp32 tensor and are folded into
# k_sb (sq*sk/sqrt(hd)) and vf_sb (sv*so) at projection-cast time.
#
# Schedule: x loads in 512-token column blocks on the Sync DMA queue while
# the weights load in parallel on the GpSimd DMA queue; the K/V/Q0
# projections for token block 0 run t-outer (one matmul per contraction
# tile as it lands) so the PE starts within a few us and the HAM clock-gate
# warms early.  The attention main loop is ACT(exp)-paced in deep query
# blocks, so all remaining projection work (Q blocks JIT, K/V for later key
# blocks, V transposes, out-proj groups, softmax normalizations) is dripped
# into it as PE filler between score/PV matmuls, paced by a per-iteration
# deficit budget so filler carries forward to the deepest (most ACT-bound)
# query blocks.  Scores are computed transposed ([key, query]) with the two
# heads of a pair on different PE row strips (concurrent); PV lags scores
# by two chunks; the rowsum rides as a ones column on V; causal masks are
# applied in-place on the exp tiles by GPSIMD affine_select.

import sys

for _p in ("/opt/trn_rl_repo",):
    if _p not in sys.path:
        sys.path.append(_p)

import numpy as np
import ml_dtypes

import concourse.bass as bass
import concourse.tile as tile
from concourse import bacc, mybir
from concourse import bass_utils
from concourse.masks import make_identity

F32 = mybir.dt.float32
BF16 = mybir.dt.bfloat16
ALU = mybir.AluOpType
ACT = mybir.ActivationFunctionType

D = 2048          # model dim
S = 2048          # sequence length
B = 2             # batch
HD = 64           # head dim
NQH = 8           # q heads per core
NKV = 2           # kv heads per core
QF = NQH * HD     # 512 q features per core
KF = NKV * HD     # 128 kv features per core
QB = 512          # query block (free dim of score matmuls)
KT = 128          # key tile (partition dim of transposed scores)
NKT = S // KT     # 16
NQB = S // QB     # 4
NDT = D // 128    # 16 contraction tiles
EPS = 1e-5

# processing order of local q heads: tile ft holds heads (ft, ft+4) so that
# the head's row block (64*(h//4)) matches its kv head's row block in k_sb.
PERM = [0, 4, 1, 5, 2, 6, 3, 7]

_NC = None
_LAST_RESULTS = None


def _build():
    nc = bacc.Bacc("TRN2", target_bir_lowering=False, debug=False, num_devices=8)

    xt_d = nc.dram_tensor("xt", [D, S], BF16, kind="ExternalInput")
    wqt_d = nc.dram_tensor("wqt", [D, QF], BF16, kind="ExternalInput")
    wkt_d = nc.dram_tensor("wkt", [D, KF], BF16, kind="ExternalInput")
    wvt_d = nc.dram_tensor("wvt", [D, KF], BF16, kind="ExternalInput")
    wot_d = nc.dram_tensor("wot", [QF, D], BF16, kind="ExternalInput")
    sc_d = nc.dram_tensor("sc", [1, 2], F32, kind="ExternalInput")
    yt_d = nc.dram_tensor("yt", [D, S], BF16, kind="ExternalOutput")

    with tile.TileContext(nc) as tc:
        with (
            tc.tile_pool(name="persist", bufs=1) as pers,
            tc.tile_pool(name="work", bufs=4) as work,
            tc.tile_pool(name="exps_p", bufs=4) as exps_p,
            tc.tile_pool(name="ysb_p", bufs=4) as ysb_p,
            tc.tile_pool(name="mm", bufs=2, space="PSUM") as mm,
            tc.tile_pool(name="scp", bufs=2, space="PSUM") as scp,
            tc.tile_pool(name="pop", bufs=2, space="PSUM") as pop,
        ):
            # ---- constants ----
            sscore_bc = pers.tile([128, 1], F32, tag="sscore")
            sout_bc = pers.tile([128, 1], F32, tag="sout")
            nc.sync.dma_start(out=sscore_bc, in_=sc_d[0:1, 0:1].to_broadcast([128, 1]))
            nc.sync.dma_start(out=sout_bc, in_=sc_d[0:1, 1:2].to_broadcast([128, 1]))
            ident = pers.tile([128, 128], BF16, tag="ident")
            make_identity(nc, ident)
            # ones row at partition 64 for the rowsum-broadcast matmul
            ones64 = pers.tile([HD + 1, HD], F32, tag="ones64")
            nc.gpsimd.memset(ones64, 1.0)

            # ---- PE warm-up ----
            # Back-to-back matmuls on the identity tile into a scratch psum
            # that is never read.  The HAM clock-gate needs ~3.4us of
            # sustained PE activity to lift the PE clock from 1.2GHz to
            # 2.4GHz; the input DMA window would otherwise leave the PE
            # sparse (and cold) for the first ~50us.  The group is left
            # open; the front-phase projection loop drips more of these
            # between DMA-paced matmuls and closes it.
            wup = scp.tile([128, 128], F32, tag="sc", name="wup")
            for i in range(40):
                nc.tensor.matmul(wup, ident, ident,
                                 start=(i == 0), stop=(i == 39))

            # ---- input DMA ----
            # Sync queue: x column blocks (in block order).  GpSimd queue:
            # weights.  The two queues issue and transfer in parallel, so
            # block 0 of x and the q/k/v weights all land within ~10us.
            wk_sb = [pers.tile([128, KF], BF16, tag=f"wk{t}", name=f"wk{t}")
                     for t in range(NDT)]
            wv_sb = [pers.tile([128, KF], BF16, tag=f"wv{t}", name=f"wv{t}")
                     for t in range(NDT)]
            wq_sb = [pers.tile([128, QF], BF16, tag=f"wq{t}", name=f"wq{t}")
                     for t in range(NDT)]
            wo_sb = [pers.tile([128, D], BF16, tag=f"wo{t}", name=f"wo{t}")
                     for t in range(QF // 128)]
            x_sb = [pers.tile([128, S], BF16, tag=f"x{t}", name=f"x{t}")
                    for t in range(NDT)]

            for t in range(NDT):
                nc.gpsimd.dma_start(out=wk_sb[t], in_=wkt_d[t * 128:(t + 1) * 128, :])
                nc.gpsimd.dma_start(out=wv_sb[t], in_=wvt_d[t * 128:(t + 1) * 128, :])
            for t in range(NDT):
                nc.sync.dma_start(out=x_sb[t][:, 0:QB],
                                  in_=xt_d[t * 128:(t + 1) * 128, 0:QB])
                nc.gpsimd.dma_start(out=wq_sb[t], in_=wqt_d[t * 128:(t + 1) * 128, :])
            for qb in (1, 2, 3):
                for t in range(NDT):
                    nc.sync.dma_start(
                        out=x_sb[t][:, qb * QB:(qb + 1) * QB],
                        in_=xt_d[t * 128:(t + 1) * 128, qb * QB:(qb + 1) * QB])
            for t in range(QF // 128):
                nc.gpsimd.dma_start(out=wo_sb[t], in_=wot_d[t * 128:(t + 1) * 128, :])

            def sign_inplace(w):
                # w <- sign(w) in {-1, +1}: (w >= 0)*2 - 1
                nc.vector.tensor_scalar(w, w, 0.0, 2.0, ALU.is_ge, ALU.mult)
                nc.vector.tensor_scalar(w, w, 1.0, None, ALU.subtract)

            for t in range(NDT):
                sign_inplace(wk_sb[t])
                sign_inplace(wv_sb[t])
            for t in range(NDT):
                sign_inplace(wq_sb[t])

            # ---- persistent activation tiles (one tile per 512-token
            # block so JIT writes and reads of different blocks are tracked
            # as distinct tensors) ----
            k_sb = [pers.tile([128, QB], BF16, tag=f"ksb{qb}", name=f"ksb{qb}")
                    for qb in range(NQB)]
            vf_sb = [pers.tile([128, QB], BF16, tag=f"vfsb{qb}",
                               name=f"vfsb{qb}") for qb in range(NQB)]
            q_sb = [[pers.tile([128, QB], BF16, tag=f"qsb{ft}_{qb}",
                               name=f"qsb{ft}_{qb}") for qb in range(NQB)]
                    for ft in range(4)]
            o_sb = [[pers.tile([128, QB], BF16, tag=f"osb{ft}_{qb}",
                               name=f"osb{ft}_{qb}") for qb in range(NQB)]
                    for ft in range(4)]
            vtok = [pers.tile([128, NKV, HD + 1], BF16, tag=f"vtok{t}",
                              name=f"vtok{t}") for t in range(NKT)]

            # ---- projection emitters (all feature-major, [feat, token]) ----
            def kv_block(qb):
                # K and V projections for token block qb, t-outer so each
                # matmul waits only on its own x tile; scales folded in at
                # the psum->sbuf cast.  Atomic: psum chains must not
                # interleave with other mm-pool allocations (FIFO deadlock).
                c0 = qb * QB
                kps = mm.tile([128, QB], F32, tag="mm", name=f"kps{qb}")
                vps = mm.tile([128, QB], F32, tag="mm", name=f"vps{qb}")
                for t in range(NDT):
                    nc.tensor.matmul(kps, wk_sb[t], x_sb[t][:, c0:c0 + QB],
                                     start=(t == 0), stop=(t == NDT - 1))
                    nc.tensor.matmul(vps, wv_sb[t], x_sb[t][:, c0:c0 + QB],
                                     start=(t == 0), stop=(t == NDT - 1))
                nc.vector.tensor_scalar(k_sb[qb], kps, sscore_bc,
                                        None, ALU.mult)
                nc.vector.tensor_scalar(vf_sb[qb], vps, sout_bc,
                                        None, ALU.mult)

            def emit_vtok(t):
                vt = vtok[t]
                pst = mm.tile([128, 128], BF16, tag="mm", name=f"vt{t}")
                nc.tensor.transpose(
                    pst, vf_sb[t // 4][:, (t % 4) * 128:(t % 4 + 1) * 128],
                    ident)
                for kv in range(NKV):
                    nc.vector.tensor_copy(vt[:, kv, 0:HD],
                                          pst[:, kv * HD:(kv + 1) * HD])
                nc.vector.memset(vt[:, :, HD:HD + 1], 1.0)

            def q_block(ft, qb):
                # Q projection for (q-tile ft, token block qb): 16-matmul
                # accumulation + cast (atomic, see kv_block).
                c0 = qb * QB
                ps = mm.tile([128, QB], F32, tag="mm", name=f"qps{ft}_{qb}")
                for t in range(NDT):
                    nc.tensor.matmul(ps, wq_sb[t][:, ft * 128:(ft + 1) * 128],
                                     x_sb[t][:, c0:c0 + QB],
                                     start=(t == 0), stop=(t == NDT - 1))
                nc.vector.tensor_copy(q_sb[ft][qb], ps)

            def emit_ygroup(qb, ot):
                # one partial out-projection psum group for query block qb
                q0 = qb * QB
                py = mm.tile([128, QB], F32, tag="mm", name=f"y{qb}_{ot}")
                for it in range(4):
                    nc.tensor.matmul(py, wo_sb[it][:, ot * 128:(ot + 1) * 128],
                                     o_sb[it][qb],
                                     start=(it == 0), stop=(it == 3))
                ysb = ysb_p.tile([128, QB], BF16, tag="ysb")
                nc.vector.tensor_copy(ysb, py)
                nc.gpsimd.dma_start(out=yt_d[ot * 128:(ot + 1) * 128, q0:q0 + QB],
                                    in_=ysb)

            # ---- front phase: K/V/Q0 projections for token block 0,
            # t-outer and interleaved so each matmul is paced by its own
            # x/wq tile DMA; then the first V transposes; wo sign-quant ----
            kps = mm.tile([128, QB], F32, tag="mm", name="kps0")
            vps = mm.tile([128, QB], F32, tag="mm", name="vps0")
            qps = scp.tile([128, QB], F32, tag="sc", name="qps00")
            for t in range(NDT):
                nc.tensor.matmul(kps, wk_sb[t], x_sb[t][:, 0:QB],
                                 start=(t == 0), stop=(t == NDT - 1))
                nc.tensor.matmul(vps, wv_sb[t], x_sb[t][:, 0:QB],
                                 start=(t == 0), stop=(t == NDT - 1))
                nc.tensor.matmul(qps, wq_sb[t][:, 0:128], x_sb[t][:, 0:QB],
                                 start=(t == 0), stop=(t == NDT - 1))
            nc.vector.tensor_scalar(k_sb[0], kps, sscore_bc, None, ALU.mult)
            nc.vector.tensor_scalar(vf_sb[0], vps, sout_bc, None, ALU.mult)
            nc.vector.tensor_copy(q_sb[0][0], qps)
            for t in range(4):
                emit_vtok(t)
            for t in range(QF // 128):
                sign_inplace(wo_sb[t])

            # ---- PE filler queue ----
            # Units are (deadline_slot, cost_ns, emit_fn); deadline_slot is
            # the linear slot index (qb*4+ft) at whose START the unit must
            # have been emitted (None = no deadline).  Dripping is paced by
            # a per-iteration credit so filler spreads into the ACT-bound
            # deep query blocks instead of draining eagerly.
            filler = []
            pending_norms = []  # normalizes of the previous slot: emitted at
            # the next slot's first iterations, before its PV matmuls need
            # the po buffers back (PE-queue order, else deadlock)
            credit = [0.0]

            def drip(budget):
                credit[0] += budget
                while filler and credit[0] > 0:
                    _, cost, fn = filler.pop(0)
                    credit[0] -= cost
                    fn()

            def drain_due(slot):
                while any(dl is not None and dl <= slot for dl, _, _ in filler):
                    filler.pop(0)[2]()
                if credit[0] > 0:
                    credit[0] = 0.0

            MM_NS = 215.0

            # ---- attention main loop ----
            for qb in range(NQB):
                # work that becomes available / needed at this query block.
                # K/V + vtok for key blocks 2 and 3 are pushed early (qb0 /
                # qb1) where the PE must stay dense to keep the HAM
                # clock-gate warm; out-proj groups for qb-1 drip during qb,
                # except 4 of qb1's reserved for qb3 (the most ACT-bound).
                if qb == 0:
                    for nxt in (1, 2):
                        filler.append((nxt * 4, 32 * MM_NS,
                                       lambda _q=nxt: kv_block(_q)))
                        filler.append((nxt * 4, 4 * 120.0, lambda _q=nxt: [
                            emit_vtok(4 * _q + i) for i in range(4)]))
                elif qb == 1:
                    filler.append((12, 32 * MM_NS, lambda: kv_block(3)))
                    filler.append((12, 4 * 120.0, lambda: [
                        emit_vtok(12 + i) for i in range(4)]))
                    filler.extend(
                        (None, 4 * MM_NS, (lambda _ot=ot: emit_ygroup(0, _ot)))
                        for ot in range(NDT))
                elif qb == 2:
                    filler.extend(
                        (None, 4 * MM_NS, (lambda _ot=ot: emit_ygroup(1, _ot)))
                        for ot in range(12))
                elif qb == 3:
                    filler.extend(
                        (None, 4 * MM_NS, (lambda _ot=ot: emit_ygroup(1, _ot)))
                        for ot in range(12, NDT))
                    filler.extend(
                        (None, 4 * MM_NS, (lambda _ot=ot: emit_ygroup(2, _ot)))
                        for ot in range(NDT))

                nkt = 4 * (qb + 1)          # causal: key tiles 0..nkt-1
                for ft in range(4):
                    slot = qb * 4 + ft
                    drain_due(slot)
                    # JIT Q projection for the next slot ((0,0) was done in
                    # the front phase).  Appended, not front-inserted: the
                    # queue must stay FIFO so psum chains never interleave.
                    if ft < 3:
                        filler.append((slot + 1, 16 * MM_NS,
                                       lambda _f=ft + 1, _q=qb: q_block(_f, _q)))
                    elif qb + 1 < NQB:
                        filler.append((slot + 1, 16 * MM_NS,
                                       lambda _q=qb + 1: q_block(0, _q)))

                    # po is allocated lazily at the first PV emission
                    # (kt=2): the deferred normalizes of the previous slot
                    # read the po buffers at kt<2, and the pool's WAR
                    # tracking only sees reads emitted before the next
                    # allocation of the tag.  Allocating early loses that
                    # edge and the next slot's PV can overwrite the rowsum
                    # row mid-read (nondeterministic corruption).
                    po_ = []

                    def emit_pv(kt, ex, _po=po_, _nkt=nkt, _qb=qb, _ft=ft):
                        if not _po:
                            _po.extend(
                                pop.tile([HD + 1, QB], F32, tag="po", bufs=2,
                                         name=f"po{_qb}_{_ft}_{p}")
                                for p in range(2))
                        for p in range(2):
                            nc.tensor.matmul(_po[p], vtok[kt][:, p, :],
                                             ex[:, p, :],
                                             start=(kt == 0),
                                             stop=(kt == _nkt - 1))

                    pend = []
                    q0 = qb * QB
                    for kt in range(nkt):
                        ps = scp.tile([128, 2, QB], F32, tag="sc", bufs=2,
                                      name=f"sc{qb}_{ft}_{kt}")
                        kb, kc = kt // 4, kt % 4
                        for p in range(2):
                            r0 = p * HD
                            nc.tensor.matmul(
                                ps[:, p, :],
                                k_sb[kb][r0:r0 + HD, kc * KT:(kc + 1) * KT],
                                q_sb[ft][qb][r0:r0 + HD, :],
                                start=True, stop=True)
                        ex = exps_p.tile([128, 2, QB], BF16, tag="ex", bufs=4,
                                         name=f"ex{qb}_{ft}_{kt}")
                        nc.scalar.activation(out=ex[:, :, :], in_=ps[:, :, :],
                                             func=ACT.Exp)
                        if kt >= 4 * qb:  # diagonal tile: causal mask,
                            # written to a separate tile (in-place RMW on a
                            # cross-engine-consumed tile is race-prone)
                            dmi = kt - 4 * qb
                            exm = exps_p.tile([128, 2, QB], BF16, tag="exm",
                                              bufs=4, name=f"exm{qb}_{ft}_{kt}")
                            for p in range(2):
                                nc.gpsimd.affine_select(
                                    out=exm[:, p, :], in_=ex[:, p, :],
                                    compare_op=ALU.is_ge, fill=0.0,
                                    base=-128 * dmi, pattern=[[1, QB]],
                                    channel_multiplier=-1)
                            ex = exm
                        pend.append((kt, ex))
                        if len(pend) > 2:
                            kt_, ex_ = pend.pop(0)
                            emit_pv(kt_, ex_)
                        # ACT-PE deficit per iteration: exp ~1110ns vs
                        # scores-pair + PV-pair ~645ns (slightly over to
                        # keep the PE ahead of the HAM idle monitor)
                        drip(520.0)
                    for kt_, ex_ in pend:
                        emit_pv(kt_, ex_)

                    # normalize: O[:, q] * (1 / rowsum[q]); rowsum is po row
                    # 64.  The rowsum row is copied to SBUF now (DVE), but
                    # the broadcast matmul + reciprocal + multiply are
                    # dripped as a filler unit so the PE does not stall on
                    # the DVE copy at the slot boundary.
                    def normalize(p, _po=po_, _ft=ft, _qb=qb):
                        rsum = work.tile([HD + 1, QB], F32, tag="rsum")
                        nc.vector.tensor_copy(rsum[HD:HD + 1, :],
                                              _po[p][HD:HD + 1, :])
                        bcp = mm.tile([HD, QB], F32, tag="mm")
                        nc.tensor.matmul(bcp, ones64[HD:HD + 1, :],
                                         rsum[HD:HD + 1, :],
                                         start=True, stop=True)
                        rbc = work.tile([HD, QB], F32, tag="rbc")
                        nc.vector.reciprocal_approx_fast(out=rbc, in_=bcp)
                        ostg = work.tile([HD, QB], BF16, tag="ostg")
                        nc.vector.tensor_tensor(ostg, _po[p][0:HD, :], rbc,
                                                ALU.mult)
                        nc.gpsimd.dma_start(
                            out=o_sb[_ft][_qb][p * HD:(p + 1) * HD, :],
                            in_=ostg)

                    for p in range(2):
                        normalize(p)

            # drain remaining filler and the last block's out-projection
            while filler:
                filler.pop(0)[2]()
            for ot in range(NDT):
                emit_ygroup(NQB - 1, ot)

    nc.compile()
    return nc


def _get_nc():
    global _NC
    if _NC is None:
        _NC = _build()
    return _NC


def run(inputs, trace=False, trace_cores=None):
    global _LAST_RESULTS
    x = np.asarray(inputs["x"], dtype=np.float32)
    wq = np.asarray(inputs["wq"], dtype=np.float32)
    wk = np.asarray(inputs["wk"], dtype=np.float32)
    wv = np.asarray(inputs["wv"], dtype=np.float32)
    wo = np.asarray(inputs["wo"], dtype=np.float32)

    sq = max(np.abs(wq).mean(), EPS)
    sk = max(np.abs(wk).mean(), EPS)
    sv = max(np.abs(wv).mean(), EPS)
    so = max(np.abs(wo).mean(), EPS)
    sc = np.array([[sq * sk / np.sqrt(HD), sv * so]], dtype=np.float32)

    perm_rows = np.concatenate([np.arange(h * HD, (h + 1) * HD) for h in PERM])

    in_maps = []
    for c in range(8):
        b, g = divmod(c, 4)
        wq_g = wq[QF * g:QF * (g + 1), :][perm_rows]        # [512, 2048]
        wk_g = wk[KF * g:KF * (g + 1), :]                   # [128, 2048]
        wv_g = wv[KF * g:KF * (g + 1), :]
        wo_g = wo[:, QF * g:QF * (g + 1)][:, perm_rows]     # [2048, 512]
        bf = ml_dtypes.bfloat16
        in_maps.append({
            "xt": np.ascontiguousarray(x[b].T).astype(bf),
            "wqt": np.ascontiguousarray(wq_g.T).astype(bf),
            "wkt": np.ascontiguousarray(wk_g.T).astype(bf),
            "wvt": np.ascontiguousarray(wv_g.T).astype(bf),
            "wot": np.ascontiguousarray(wo_g.T).astype(bf),
            "sc": sc,
        })

    nc = _get_nc()
    kwargs = {}
    if trace:
        kwargs["trace"] = True
        kwargs["trace_cores"] = trace_cores if trace_cores is not None else [0]
    res = bass_utils.run_bass_kernel_spmd(nc, in_maps, list(range(8)), **kwargs)
    _LAST_RESULTS = res

    y = np.empty((B, S, D), dtype=np.float32)
    for b in range(B):
        acc = np.zeros((D, S), dtype=np.float32)
        for g in range(4):
            acc += res.results[4 * b + g]["yt"].astype(np.float32)
        y[b] = acc.T
    return y


def kernel(**inputs):
    return run(inputs, trace=False)


# revision 51
# speedup vs baseline: 1.0633x; 1.0633x over previous
# BitAttention (ternary-quantized GQA transformer block) on 8 Trainium2 NeuronCores.
#
# Reference computation (see problem):
#   w_q = sign(w) * mean(|w|)            (per weight tensor, global scale)
#   q = x @ w_q(wq).T ; k = x @ w_q(wk).T ; v = x @ w_q(wv).T
#   GQA causal attention (32 q heads, 8 kv heads, head_dim 64)
#   out = attn @ w_q(wo).T
#
# Sharding (8 cores): batch (2) x kv-head-group (4).  Each core computes
# attention for 2 kv heads / 8 q heads of one batch and a partial out-proj
# over its 512 attention-output features; the host sums 4 partials per batch.
#
# Device layout: activations are feature-major ("transposed", [feat, token]).
# Inputs enter pre-transposed/sliced in bf16; sign() is computed on device;
# the quant scales enter as a tiny [1,2] fp32 tensor and are folded into
# k_sb (sq*sk/sqrt(hd)) and vf_sb (sv*so) at projection-cast time.
#
# Schedule: x loads in 512-token column blocks on the Sync DMA queue while
# the weights load in parallel on the GpSimd DMA queue; the K/V/Q0
# projections for token block 0 run t-outer (one matmul per contraction
# tile as it lands) so the PE starts within a few us and the HAM clock-gate
# warms early.  The attention main loop is ACT(exp)-paced in deep query
# blocks, so all remaining projection work (Q blocks JIT, K/V for later key
# blocks, V transposes, out-proj groups, softmax normalizations) is dripped
# into it as PE filler between score/PV matmuls, paced by a per-iteration
# deficit budget so filler carries forward to the deepest (most ACT-bound)
# query blocks.  Scores are computed transposed ([key, query]) with the two
# heads of a pair on different PE row strips (concurrent); PV lags scores
# by two chunks; the rowsum rides as a ones column on V; causal masks are
# applied in-place on the exp tiles by GPSIMD affine_select.

import sys

for _p in ("/opt/trn_rl_repo",):
    if _p not in sys.path:
        sys.path.append(_p)

import numpy as np
import ml_dtypes

import concourse.bass as bass
import concourse.tile as tile
from concourse import bacc, mybir
from concourse import bass_utils
from concourse.masks import make_identity

F32 = mybir.dt.float32
BF16 = mybir.dt.bfloat16
ALU = mybir.AluOpType
ACT = mybir.ActivationFunctionType

D = 2048          # model dim
S = 2048          # sequence length
B = 2             # batch
HD = 64           # head dim
NQH = 8           # q heads per core
NKV = 2           # kv heads per core
QF = NQH * HD     # 512 q features per core
KF = NKV * HD     # 128 kv features per core
QB = 512          # query block (free dim of score matmuls)
KT = 128          # key tile (partition dim of transposed scores)
NKT = S // KT     # 16
NQB = S // QB     # 4
NDT = D // 128    # 16 contraction tiles
EPS = 1e-5

# processing order of local q heads: tile ft holds heads (ft, ft+4) so that
# the head's row block (64*(h//4)) matches its kv head's row block in k_sb.
PERM = [0, 4, 1, 5, 2, 6, 3, 7]

_NC = None
_LAST_RESULTS = None


def _build():
    nc = bacc.Bacc("TRN2", target_bir_lowering=False, debug=False, num_devices=8)

    xt_d = nc.dram_tensor("xt", [D, S], BF16, kind="ExternalInput")
    wqt_d = nc.dram_tensor("wqt", [D, QF], BF16, kind="ExternalInput")
    wkt_d = nc.dram_tensor("wkt", [D, KF], BF16, kind="ExternalInput")
    wvt_d = nc.dram_tensor("wvt", [D, KF], BF16, kind="ExternalInput")
    wot_d = nc.dram_tensor("wot", [QF, D], BF16, kind="ExternalInput")
    sc_d = nc.dram_tensor("sc", [1, 2], F32, kind="ExternalInput")
    yt_d = nc.dram_tensor("yt", [D, S], BF16, kind="ExternalOutput")
    # scratch rows for the rowsum-reciprocal partition broadcast: DMA can
    # broadcast a DRAM row across partitions but not an SBUF row.  Write
    # and broadcast-read go through the same (Sync) DMA queue, whose FIFO
    # guarantees the ordering.
    rs_d = nc.dram_tensor("rs_scratch", [2 * NQB * 4, QB], F32, kind="Internal")

    with tile.TileContext(nc) as tc:
        with (
            tc.tile_pool(name="persist", bufs=1) as pers,
            tc.tile_pool(name="work", bufs=4) as work,
            tc.tile_pool(name="exps_p", bufs=4) as exps_p,
            tc.tile_pool(name="ysb_p", bufs=4) as ysb_p,
            tc.tile_pool(name="mm", bufs=2, space="PSUM") as mm,
            tc.tile_pool(name="scp", bufs=2, space="PSUM") as scp,
            tc.tile_pool(name="pop", bufs=2, space="PSUM") as pop,
        ):
            # ---- constants ----
            sscore_bc = pers.tile([128, 1], F32, tag="sscore")
            sout_bc = pers.tile([128, 1], F32, tag="sout")
            nc.sync.dma_start(out=sscore_bc, in_=sc_d[0:1, 0:1].to_broadcast([128, 1]))
            nc.sync.dma_start(out=sout_bc, in_=sc_d[0:1, 1:2].to_broadcast([128, 1]))
            ident = pers.tile([128, 128], BF16, tag="ident")
            make_identity(nc, ident)
            # ones row at partition 64 for the rowsum-broadcast matmul
            ones64 = pers.tile([HD + 1, HD], F32, tag="ones64")
            nc.gpsimd.memset(ones64, 1.0)

            # ---- PE warm-up ----
            # Back-to-back matmuls on the identity tile into a scratch psum
            # that is never read.  The HAM clock-gate needs ~3.4us of
            # sustained PE activity to lift the PE clock from 1.2GHz to
            # 2.4GHz; the input DMA window would otherwise leave the PE
            # sparse (and cold) for the first ~50us.  The group is left
            # open; the front-phase projection loop drips more of these
            # between DMA-paced matmuls and closes it.
            wup = scp.tile([128, 128], F32, tag="sc", name="wup")
            for i in range(40):
                nc.tensor.matmul(wup, ident, ident,
                                 start=(i == 0), stop=(i == 39))

            # ---- input DMA ----
            # Sync queue: x column blocks (in block order).  GpSimd queue:
            # weights.  The two queues issue and transfer in parallel, so
            # block 0 of x and the q/k/v weights all land within ~10us.
            wk_sb = [pers.tile([128, KF], BF16, tag=f"wk{t}", name=f"wk{t}")
                     for t in range(NDT)]
            wv_sb = [pers.tile([128, KF], BF16, tag=f"wv{t}", name=f"wv{t}")
                     for t in range(NDT)]
            wq_sb = [pers.tile([128, QF], BF16, tag=f"wq{t}", name=f"wq{t}")
                     for t in range(NDT)]
            wo_sb = [pers.tile([128, D], BF16, tag=f"wo{t}", name=f"wo{t}")
                     for t in range(QF // 128)]
            x_sb = [pers.tile([128, S], BF16, tag=f"x{t}", name=f"x{t}")
                    for t in range(NDT)]

            for t in range(NDT):
                nc.gpsimd.dma_start(out=wk_sb[t], in_=wkt_d[t * 128:(t + 1) * 128, :])
                nc.gpsimd.dma_start(out=wv_sb[t], in_=wvt_d[t * 128:(t + 1) * 128, :])
            for t in range(NDT):
                nc.sync.dma_start(out=x_sb[t][:, 0:QB],
                                  in_=xt_d[t * 128:(t + 1) * 128, 0:QB])
                nc.gpsimd.dma_start(out=wq_sb[t], in_=wqt_d[t * 128:(t + 1) * 128, :])
            for qb in (1, 2, 3):
                for t in range(NDT):
                    nc.sync.dma_start(
                        out=x_sb[t][:, qb * QB:(qb + 1) * QB],
                        in_=xt_d[t * 128:(t + 1) * 128, qb * QB:(qb + 1) * QB])
            for t in range(QF // 128):
                nc.gpsimd.dma_start(out=wo_sb[t], in_=wot_d[t * 128:(t + 1) * 128, :])

            def sign_inplace(w):
                # w <- sign(w) in {-1, +1}: (w >= 0)*2 - 1
                nc.vector.tensor_scalar(w, w, 0.0, 2.0, ALU.is_ge, ALU.mult)
                nc.vector.tensor_scalar(w, w, 1.0, None, ALU.subtract)

            for t in range(NDT):
                sign_inplace(wk_sb[t])
                sign_inplace(wv_sb[t])
            for t in range(NDT):
                sign_inplace(wq_sb[t])

            # ---- persistent activation tiles (one tile per 512-token
            # block so JIT writes and reads of different blocks are tracked
            # as distinct tensors) ----
            k_sb = [pers.tile([128, QB], BF16, tag=f"ksb{qb}", name=f"ksb{qb}")
                    for qb in range(NQB)]
            vf_sb = [pers.tile([128, QB], BF16, tag=f"vfsb{qb}",
                               name=f"vfsb{qb}") for qb in range(NQB)]
            q_sb = [[pers.tile([128, QB], BF16, tag=f"qsb{ft}_{qb}",
                               name=f"qsb{ft}_{qb}") for qb in range(NQB)]
                    for ft in range(4)]
            o_sb = [[pers.tile([128, QB], BF16, tag=f"osb{ft}_{qb}",
                               name=f"osb{ft}_{qb}") for qb in range(NQB)]
                    for ft in range(4)]
            vtok = [pers.tile([128, NKV, HD + 1], BF16, tag=f"vtok{t}",
                              name=f"vtok{t}") for t in range(NKT)]

            # ---- projection emitters (all feature-major, [feat, token]) ----
            def kv_block(qb):
                # K and V projections for token block qb, t-outer so each
                # matmul waits only on its own x tile; scales folded in at
                # the psum->sbuf cast.  Atomic: psum chains must not
                # interleave with other mm-pool allocations (FIFO deadlock).
                c0 = qb * QB
                kps = mm.tile([128, QB], F32, tag="mm", name=f"kps{qb}")
                vps = mm.tile([128, QB], F32, tag="mm", name=f"vps{qb}")
                for t in range(NDT):
                    nc.tensor.matmul(kps, wk_sb[t], x_sb[t][:, c0:c0 + QB],
                                     start=(t == 0), stop=(t == NDT - 1))
                    nc.tensor.matmul(vps, wv_sb[t], x_sb[t][:, c0:c0 + QB],
                                     start=(t == 0), stop=(t == NDT - 1))
                nc.vector.tensor_scalar(k_sb[qb], kps, sscore_bc,
                                        None, ALU.mult)
                nc.vector.tensor_scalar(vf_sb[qb], vps, sout_bc,
                                        None, ALU.mult)

            def emit_vtok(t):
                vt = vtok[t]
                pst = mm.tile([128, 128], BF16, tag="mm", name=f"vt{t}")
                nc.tensor.transpose(
                    pst, vf_sb[t // 4][:, (t % 4) * 128:(t % 4 + 1) * 128],
                    ident)
                for kv in range(NKV):
                    nc.vector.tensor_copy(vt[:, kv, 0:HD],
                                          pst[:, kv * HD:(kv + 1) * HD])
                nc.vector.memset(vt[:, :, HD:HD + 1], 1.0)

            def q_block(ft, qb):
                # Q projection for (q-tile ft, token block qb): 16-matmul
                # accumulation + cast (atomic, see kv_block).
                c0 = qb * QB
                ps = mm.tile([128, QB], F32, tag="mm", name=f"qps{ft}_{qb}")
                for t in range(NDT):
                    nc.tensor.matmul(ps, wq_sb[t][:, ft * 128:(ft + 1) * 128],
                                     x_sb[t][:, c0:c0 + QB],
                                     start=(t == 0), stop=(t == NDT - 1))
                nc.vector.tensor_copy(q_sb[ft][qb], ps)

            def emit_ygroup(qb, ot):
                # one partial out-projection psum group for query block qb
                q0 = qb * QB
                py = mm.tile([128, QB], F32, tag="mm", name=f"y{qb}_{ot}")
                for it in range(4):
                    nc.tensor.matmul(py, wo_sb[it][:, ot * 128:(ot + 1) * 128],
                                     o_sb[it][qb],
                                     start=(it == 0), stop=(it == 3))
                ysb = ysb_p.tile([128, QB], BF16, tag="ysb")
                nc.vector.tensor_copy(ysb, py)
                nc.gpsimd.dma_start(out=yt_d[ot * 128:(ot + 1) * 128, q0:q0 + QB],
                                    in_=ysb)

            # ---- front phase: K/V/Q0 projections for token block 0,
            # t-outer and interleaved so each matmul is paced by its own
            # x/wq tile DMA; then the first V transposes; wo sign-quant ----
            kps = mm.tile([128, QB], F32, tag="mm", name="kps0")
            vps = mm.tile([128, QB], F32, tag="mm", name="vps0")
            qps = scp.tile([128, QB], F32, tag="sc", name="qps00")
            for t in range(NDT):
                nc.tensor.matmul(kps, wk_sb[t], x_sb[t][:, 0:QB],
                                 start=(t == 0), stop=(t == NDT - 1))
                nc.tensor.matmul(vps, wv_sb[t], x_sb[t][:, 0:QB],
                                 start=(t == 0), stop=(t == NDT - 1))
                nc.tensor.matmul(qps, wq_sb[t][:, 0:128], x_sb[t][:, 0:QB],
                                 start=(t == 0), stop=(t == NDT - 1))
            nc.vector.tensor_scalar(k_sb[0], kps, sscore_bc, None, ALU.mult)
            nc.vector.tensor_scalar(vf_sb[0], vps, sout_bc, None, ALU.mult)
            nc.vector.tensor_copy(q_sb[0][0], qps)
            for t in range(4):
                emit_vtok(t)
            for t in range(QF // 128):
                sign_inplace(wo_sb[t])

            # ---- PE filler queue ----
            # Units are (deadline_slot, cost_ns, emit_fn); deadline_slot is
            # the linear slot index (qb*4+ft) at whose START the unit must
            # have been emitted (None = no deadline).  Dripping is paced by
            # a per-iteration credit so filler spreads into the ACT-bound
            # deep query blocks instead of draining eagerly.
            filler = []
            pending_norms = []  # normalizes of the previous slot: emitted at
            # the next slot's first iterations, before its PV matmuls need
            # the po buffers back (PE-queue order, else deadlock)
            credit = [0.0]

            def drip(budget):
                credit[0] += budget
                while filler and credit[0] > 0:
                    _, cost, fn = filler.pop(0)
                    credit[0] -= cost
                    fn()

            def drain_due(slot):
                while any(dl is not None and dl <= slot for dl, _, _ in filler):
                    filler.pop(0)[2]()
                if credit[0] > 0:
                    credit[0] = 0.0

            MM_NS = 215.0

            # ---- attention main loop ----
            for qb in range(NQB):
                # work that becomes available / needed at this query block.
                # K/V + vtok for key blocks 2 and 3 are pushed early (qb0 /
                # qb1) where the PE must stay dense to keep the HAM
                # clock-gate warm; out-proj groups for qb-1 drip during qb,
                # except 4 of qb1's reserved for qb3 (the most ACT-bound).
                if qb == 0:
                    for nxt in (1, 2):
                        filler.append((nxt * 4, 32 * MM_NS,
                                       lambda _q=nxt: kv_block(_q)))
                        filler.append((nxt * 4, 4 * 120.0, lambda _q=nxt: [
                            emit_vtok(4 * _q + i) for i in range(4)]))
                elif qb == 1:
                    filler.append((12, 32 * MM_NS, lambda: kv_block(3)))
                    filler.append((12, 4 * 120.0, lambda: [
                        emit_vtok(12 + i) for i in range(4)]))
                    filler.extend(
                        (None, 4 * MM_NS, (lambda _ot=ot: emit_ygroup(0, _ot)))
                        for ot in range(NDT))
                elif qb == 2:
                    filler.extend(
                        (None, 4 * MM_NS, (lambda _ot=ot: emit_ygroup(1, _ot)))
                        for ot in range(12))
                elif qb == 3:
                    filler.extend(
                        (None, 4 * MM_NS, (lambda _ot=ot: emit_ygroup(1, _ot)))
                        for ot in range(12, NDT))
                    filler.extend(
                        (None, 4 * MM_NS, (lambda _ot=ot: emit_ygroup(2, _ot)))
                        for ot in range(NDT))

                nkt = 4 * (qb + 1)          # causal: key tiles 0..nkt-1
                for ft in range(4):
                    slot = qb * 4 + ft
                    drain_due(slot)
                    # JIT Q projection for the next slot ((0,0) was done in
                    # the front phase).  Appended, not front-inserted: the
                    # queue must stay FIFO so psum chains never interleave.
                    if ft < 3:
                        filler.append((slot + 1, 16 * MM_NS,
                                       lambda _f=ft + 1, _q=qb: q_block(_f, _q)))
                    elif qb + 1 < NQB:
                        filler.append((slot + 1, 16 * MM_NS,
                                       lambda _q=qb + 1: q_block(0, _q)))

                    # po is allocated lazily at the first PV emission
                    # (kt=2): the deferred normalizes of the previous slot
                    # read the po buffers at kt<2, and the pool's WAR
                    # tracking only sees reads emitted before the next
                    # allocation of the tag.  Allocating early loses that
                    # edge and the next slot's PV can overwrite the rowsum
                    # row mid-read (nondeterministic corruption).
                    po_ = []

                    def emit_pv(kt, ex, _po=po_, _nkt=nkt, _qb=qb, _ft=ft):
                        if not _po:
                            _po.extend(
                                pop.tile([HD + 1, QB], F32, tag="po", bufs=2,
                                         name=f"po{_qb}_{_ft}_{p}")
                                for p in range(2))
                        for p in range(2):
                            nc.tensor.matmul(_po[p], vtok[kt][:, p, :],
                                             ex[:, p, :],
                                             start=(kt == 0),
                                             stop=(kt == _nkt - 1))

                    pend = []
                    q0 = qb * QB
                    for kt in range(nkt):
                        ps = scp.tile([128, 2, QB], F32, tag="sc", bufs=2,
                                      name=f"sc{qb}_{ft}_{kt}")
                        kb, kc = kt // 4, kt % 4
                        for p in range(2):
                            r0 = p * HD
                            nc.tensor.matmul(
                                ps[:, p, :],
                                k_sb[kb][r0:r0 + HD, kc * KT:(kc + 1) * KT],
                                q_sb[ft][qb][r0:r0 + HD, :],
                                start=True, stop=True)
                        ex = exps_p.tile([128, 2, QB], BF16, tag="ex", bufs=4,
                                         name=f"ex{qb}_{ft}_{kt}")
                        nc.scalar.activation(out=ex[:, :, :], in_=ps[:, :, :],
                                             func=ACT.Exp)
                        if kt >= 4 * qb:  # diagonal tile: causal mask,
                            # written to a separate tile (in-place RMW on a
                            # cross-engine-consumed tile is race-prone)
                            dmi = kt - 4 * qb
                            exm = exps_p.tile([128, 2, QB], BF16, tag="exm",
                                              bufs=4, name=f"exm{qb}_{ft}_{kt}")
                            for p in range(2):
                                nc.gpsimd.affine_select(
                                    out=exm[:, p, :], in_=ex[:, p, :],
                                    compare_op=ALU.is_ge, fill=0.0,
                                    base=-128 * dmi, pattern=[[1, QB]],
                                    channel_multiplier=-1)
                            ex = exm
                        pend.append((kt, ex))
                        if len(pend) > 2:
                            kt_, ex_ = pend.pop(0)
                            emit_pv(kt_, ex_)
                        # ACT-PE deficit per iteration: exp ~1110ns vs
                        # scores-pair + PV-pair ~645ns (slightly over to
                        # keep the PE ahead of the HAM idle monitor)
                        drip(520.0)
                    for kt_, ex_ in pend:
                        emit_pv(kt_, ex_)

                    # normalize: O[:, q] * (1 / rowsum[q]); rowsum is po row
                    # 64.  The rowsum row is copied to SBUF now (DVE), but
                    # the broadcast matmul + reciprocal + multiply are
                    # dripped as a filler unit so the PE does not stall on
                    # the DVE copy at the slot boundary.
                    def normalize(p, _po=po_, _ft=ft, _qb=qb, _slot=slot):
                        rid = _slot * 2 + p
                        rsum = work.tile([HD + 1, QB], F32, tag="rsum")
                        nc.vector.tensor_copy(rsum[HD:HD + 1, :],
                                              _po[p][HD:HD + 1, :])
                        nc.sync.dma_start(out=rs_d[rid:rid + 1, :],
                                          in_=rsum[HD:HD + 1, :])
                        rbc = work.tile([HD, QB], F32, tag="rbc")
                        nc.sync.dma_start(
                            out=rbc,
                            in_=rs_d[rid:rid + 1, :].to_broadcast([HD, QB]))
                        bcr = work.tile([HD, QB], F32, tag="bcr")
                        nc.vector.reciprocal_approx_fast(out=bcr, in_=rbc)
                        ostg = work.tile([HD, QB], BF16, tag="ostg")
                        nc.vector.tensor_tensor(ostg, _po[p][0:HD, :], bcr,
                                                ALU.mult)
                        nc.gpsimd.dma_start(
                            out=o_sb[_ft][_qb][p * HD:(p + 1) * HD, :],
                            in_=ostg)

                    for p in range(2):
                        normalize(p)

            # drain remaining filler and the last block's out-projection
            while filler:
                filler.pop(0)[2]()
            for ot in range(NDT):
                emit_ygroup(NQB - 1, ot)

    nc.compile()
    return nc


def _get_nc():
    global _NC
    if _NC is None:
        _NC = _build()
    return _NC


def run(inputs, trace=False, trace_cores=None):
    global _LAST_RESULTS
    x = np.asarray(inputs["x"], dtype=np.float32)
    wq = np.asarray(inputs["wq"], dtype=np.float32)
    wk = np.asarray(inputs["wk"], dtype=np.float32)
    wv = np.asarray(inputs["wv"], dtype=np.float32)
    wo = np.asarray(inputs["wo"], dtype=np.float32)

    sq = max(np.abs(wq).mean(), EPS)
    sk = max(np.abs(wk).mean(), EPS)
    sv = max(np.abs(wv).mean(), EPS)
    so = max(np.abs(wo).mean(), EPS)
    sc = np.array([[sq * sk / np.sqrt(HD), sv * so]], dtype=np.float32)

    perm_rows = np.concatenate([np.arange(h * HD, (h + 1) * HD) for h in PERM])

    in_maps = []
    for c in range(8):
        b, g = divmod(c, 4)
        wq_g = wq[QF * g:QF * (g + 1), :][perm_rows]        # [512, 2048]
        wk_g = wk[KF * g:KF * (g + 1), :]                   # [128, 2048]
        wv_g = wv[KF * g:KF * (g + 1), :]
        wo_g = wo[:, QF * g:QF * (g + 1)][:, perm_rows]     # [2048, 512]
        bf = ml_dtypes.bfloat16
        in_maps.append({
            "xt": np.ascontiguousarray(x[b].T).astype(bf),
            "wqt": np.ascontiguousarray(wq_g.T).astype(bf),
            "wkt": np.ascontiguousarray(wk_g.T).astype(bf),
            "wvt": np.ascontiguousarray(wv_g.T).astype(bf),
            "wot": np.ascontiguousarray(wo_g.T).astype(bf),
            "sc": sc,
        })

    nc = _get_nc()
    kwargs = {}
    if trace:
        kwargs["trace"] = True
        kwargs["trace_cores"] = trace_cores if trace_cores is not None else [0]
    res = bass_utils.run_bass_kernel_spmd(nc, in_maps, list(range(8)), **kwargs)
    _LAST_RESULTS = res

    y = np.empty((B, S, D), dtype=np.float32)
    for b in range(B):
        acc = np.zeros((D, S), dtype=np.float32)
        for g in range(4):
            acc += res.results[4 * b + g]["yt"].astype(np.float32)
        y[b] = acc.T
    return y


def kernel(**inputs):
    return run(inputs, trace=False)


# revision 52
# speedup vs baseline: 1.0862x; 1.0216x over previous
# BitAttention (ternary-quantized GQA transformer block) on 8 Trainium2 NeuronCores.
#
# Reference computation (see problem):
#   w_q = sign(w) * mean(|w|)            (per weight tensor, global scale)
#   q = x @ w_q(wq).T ; k = x @ w_q(wk).T ; v = x @ w_q(wv).T
#   GQA causal attention (32 q heads, 8 kv heads, head_dim 64)
#   out = attn @ w_q(wo).T
#
# Sharding (8 cores): batch (2) x kv-head-group (4).  Each core computes
# attention for 2 kv heads / 8 q heads of one batch and a partial out-proj
# over its 512 attention-output features; the host sums 4 partials per batch.
#
# Device layout: activations are feature-major ("transposed", [feat, token]).
# Inputs enter pre-transposed/sliced in bf16; sign() is computed on device;
# the quant scales enter as a tiny [1,2] fp32 tensor and are folded into
# k_sb (sq*sk/sqrt(hd)) and vf_sb (sv*so) at projection-cast time.
#
# Schedule: x loads in 512-token column blocks on the Sync DMA queue while
# the weights load in parallel on the GpSimd DMA queue; the K/V/Q0
# projections for token block 0 run t-outer (one matmul per contraction
# tile as it lands) so the PE starts within a few us and the HAM clock-gate
# warms early.  The attention main loop is ACT(exp)-paced in deep query
# blocks, so all remaining projection work (Q blocks JIT, K/V for later key
# blocks, V transposes, out-proj groups, softmax normalizations) is dripped
# into it as PE filler between score/PV matmuls, paced by a per-iteration
# deficit budget so filler carries forward to the deepest (most ACT-bound)
# query blocks.  Scores are computed transposed ([key, query]) with the two
# heads of a pair on different PE row strips (concurrent); PV lags scores
# by two chunks; the rowsum rides as a ones column on V; causal masks are
# applied in-place on the exp tiles by GPSIMD affine_select.

import sys

for _p in ("/opt/trn_rl_repo",):
    if _p not in sys.path:
        sys.path.append(_p)

import numpy as np
import ml_dtypes

import concourse.bass as bass
import concourse.tile as tile
from concourse import bacc, mybir
from concourse import bass_utils
from concourse.masks import make_identity

F32 = mybir.dt.float32
BF16 = mybir.dt.bfloat16
ALU = mybir.AluOpType
ACT = mybir.ActivationFunctionType

D = 2048          # model dim
S = 2048          # sequence length
B = 2             # batch
HD = 64           # head dim
NQH = 8           # q heads per core
NKV = 2           # kv heads per core
QF = NQH * HD     # 512 q features per core
KF = NKV * HD     # 128 kv features per core
QB = 512          # query block (free dim of score matmuls)
KT = 128          # key tile (partition dim of transposed scores)
NKT = S // KT     # 16
NQB = S // QB     # 4
NDT = D // 128    # 16 contraction tiles
EPS = 1e-5

# processing order of local q heads: tile ft holds heads (ft, ft+4) so that
# the head's row block (64*(h//4)) matches its kv head's row block in k_sb.
PERM = [0, 4, 1, 5, 2, 6, 3, 7]

_NC = None
_LAST_RESULTS = None


def _build():
    nc = bacc.Bacc("TRN2", target_bir_lowering=False, debug=False, num_devices=8)

    xt_d = nc.dram_tensor("xt", [D, S], BF16, kind="ExternalInput")
    wqt_d = nc.dram_tensor("wqt", [D, QF], BF16, kind="ExternalInput")
    wkt_d = nc.dram_tensor("wkt", [D, KF], BF16, kind="ExternalInput")
    wvt_d = nc.dram_tensor("wvt", [D, KF], BF16, kind="ExternalInput")
    wot_d = nc.dram_tensor("wot", [QF, D], BF16, kind="ExternalInput")
    sc_d = nc.dram_tensor("sc", [1, 2], F32, kind="ExternalInput")
    yt_d = nc.dram_tensor("yt", [D, S], BF16, kind="ExternalOutput")
    # scratch rows for the rowsum-reciprocal partition broadcast: DMA can
    # broadcast a DRAM row across partitions but not an SBUF row.  Write
    # and broadcast-read go through the same (Sync) DMA queue, whose FIFO
    # guarantees the ordering.
    rs_d = nc.dram_tensor("rs_scratch", [2 * NQB * 4, QB], F32, kind="Internal")

    with tile.TileContext(nc) as tc:
        with (
            tc.tile_pool(name="persist", bufs=1) as pers,
            tc.tile_pool(name="work", bufs=4) as work,
            tc.tile_pool(name="exps_p", bufs=4) as exps_p,
            tc.tile_pool(name="ysb_p", bufs=4) as ysb_p,
            tc.tile_pool(name="mm", bufs=2, space="PSUM") as mm,
            tc.tile_pool(name="scp", bufs=2, space="PSUM") as scp,
            tc.tile_pool(name="pop", bufs=2, space="PSUM") as pop,
        ):
            # ---- constants ----
            sscore_bc = pers.tile([128, 1], F32, tag="sscore")
            sout_bc = pers.tile([128, 1], F32, tag="sout")
            nc.sync.dma_start(out=sscore_bc, in_=sc_d[0:1, 0:1].to_broadcast([128, 1]))
            nc.sync.dma_start(out=sout_bc, in_=sc_d[0:1, 1:2].to_broadcast([128, 1]))
            ident = pers.tile([128, 128], BF16, tag="ident")
            make_identity(nc, ident)
            # ones row at partition 64 for the rowsum-broadcast matmul
            ones64 = pers.tile([HD + 1, HD], F32, tag="ones64")
            nc.gpsimd.memset(ones64, 1.0)

            # ---- PE warm-up ----
            # Back-to-back matmuls on the identity tile into a scratch psum
            # that is never read.  The HAM clock-gate needs ~3.4us of
            # sustained PE activity to lift the PE clock from 1.2GHz to
            # 2.4GHz; the input DMA window would otherwise leave the PE
            # sparse (and cold) for the first ~50us.  The group is left
            # open; the front-phase projection loop drips more of these
            # between DMA-paced matmuls and closes it.
            wup = scp.tile([128, 128], F32, tag="sc", name="wup")
            for i in range(40):
                nc.tensor.matmul(wup, ident, ident,
                                 start=(i == 0), stop=(i == 39))

            # ---- input DMA ----
            # Sync queue: x column blocks (in block order).  GpSimd queue:
            # weights.  The two queues issue and transfer in parallel, so
            # block 0 of x and the q/k/v weights all land within ~10us.
            wk_sb = [pers.tile([128, KF], BF16, tag=f"wk{t}", name=f"wk{t}")
                     for t in range(NDT)]
            wv_sb = [pers.tile([128, KF], BF16, tag=f"wv{t}", name=f"wv{t}")
                     for t in range(NDT)]
            wq_sb = [pers.tile([128, QF], BF16, tag=f"wq{t}", name=f"wq{t}")
                     for t in range(NDT)]
            wo_sb = [pers.tile([128, D], BF16, tag=f"wo{t}", name=f"wo{t}")
                     for t in range(QF // 128)]
            x_sb = [pers.tile([128, S], BF16, tag=f"x{t}", name=f"x{t}")
                    for t in range(NDT)]

            for t in range(NDT):
                nc.gpsimd.dma_start(out=wk_sb[t], in_=wkt_d[t * 128:(t + 1) * 128, :])
                nc.gpsimd.dma_start(out=wv_sb[t], in_=wvt_d[t * 128:(t + 1) * 128, :])
            for t in range(NDT):
                nc.sync.dma_start(out=x_sb[t][:, 0:QB],
                                  in_=xt_d[t * 128:(t + 1) * 128, 0:QB])
                nc.gpsimd.dma_start(out=wq_sb[t], in_=wqt_d[t * 128:(t + 1) * 128, :])
            for qb in (1, 2, 3):
                for t in range(NDT):
                    nc.sync.dma_start(
                        out=x_sb[t][:, qb * QB:(qb + 1) * QB],
                        in_=xt_d[t * 128:(t + 1) * 128, qb * QB:(qb + 1) * QB])
            for t in range(QF // 128):
                nc.gpsimd.dma_start(out=wo_sb[t], in_=wot_d[t * 128:(t + 1) * 128, :])

            def sign_inplace(w):
                # w <- sign(w) in {-1, +1}: (w >= 0)*2 - 1
                nc.vector.tensor_scalar(w, w, 0.0, 2.0, ALU.is_ge, ALU.mult)
                nc.vector.tensor_scalar(w, w, 1.0, None, ALU.subtract)

            for t in range(NDT):
                sign_inplace(wk_sb[t])
                sign_inplace(wv_sb[t])
            for t in range(NDT):
                sign_inplace(wq_sb[t])

            # ---- persistent activation tiles (one tile per 512-token
            # block so JIT writes and reads of different blocks are tracked
            # as distinct tensors) ----
            k_sb = [pers.tile([128, QB], BF16, tag=f"ksb{qb}", name=f"ksb{qb}")
                    for qb in range(NQB)]
            vf_sb = [pers.tile([128, QB], BF16, tag=f"vfsb{qb}",
                               name=f"vfsb{qb}") for qb in range(NQB)]
            q_sb = [[pers.tile([128, QB], BF16, tag=f"qsb{ft}_{qb}",
                               name=f"qsb{ft}_{qb}") for qb in range(NQB)]
                    for ft in range(4)]
            o_sb = [[pers.tile([128, QB], BF16, tag=f"osb{ft}_{qb}",
                               name=f"osb{ft}_{qb}") for qb in range(NQB)]
                    for ft in range(4)]
            vtok = [pers.tile([128, NKV, HD + 1], BF16, tag=f"vtok{t}",
                              name=f"vtok{t}") for t in range(NKT)]

            # ---- projection emitters (all feature-major, [feat, token]) ----
            def kv_block(qb):
                # K and V projections for token block qb, t-outer so each
                # matmul waits only on its own x tile; scales folded in at
                # the psum->sbuf cast.  Atomic: psum chains must not
                # interleave with other mm-pool allocations (FIFO deadlock).
                c0 = qb * QB
                kps = mm.tile([128, QB], F32, tag="mm", name=f"kps{qb}")
                vps = mm.tile([128, QB], F32, tag="mm", name=f"vps{qb}")
                for t in range(NDT):
                    nc.tensor.matmul(kps, wk_sb[t], x_sb[t][:, c0:c0 + QB],
                                     start=(t == 0), stop=(t == NDT - 1))
                    nc.tensor.matmul(vps, wv_sb[t], x_sb[t][:, c0:c0 + QB],
                                     start=(t == 0), stop=(t == NDT - 1))
                nc.vector.tensor_scalar(k_sb[qb], kps, sscore_bc,
                                        None, ALU.mult)
                nc.vector.tensor_scalar(vf_sb[qb], vps, sout_bc,
                                        None, ALU.mult)

            def emit_vtok(t):
                vt = vtok[t]
                pst = mm.tile([128, 128], BF16, tag="mm", name=f"vt{t}")
                nc.tensor.transpose(
                    pst, vf_sb[t // 4][:, (t % 4) * 128:(t % 4 + 1) * 128],
                    ident)
                for kv in range(NKV):
                    nc.vector.tensor_copy(vt[:, kv, 0:HD],
                                          pst[:, kv * HD:(kv + 1) * HD])
                nc.vector.memset(vt[:, :, HD:HD + 1], 1.0)

            def q_block(ft, qb):
                # Q projection for (q-tile ft, token block qb): 16-matmul
                # accumulation + cast (atomic, see kv_block).
                c0 = qb * QB
                ps = mm.tile([128, QB], F32, tag="mm", name=f"qps{ft}_{qb}")
                for t in range(NDT):
                    nc.tensor.matmul(ps, wq_sb[t][:, ft * 128:(ft + 1) * 128],
                                     x_sb[t][:, c0:c0 + QB],
                                     start=(t == 0), stop=(t == NDT - 1))
                nc.vector.tensor_copy(q_sb[ft][qb], ps)

            def emit_ygroup(qb, ot):
                # one partial out-projection psum group for query block qb
                q0 = qb * QB
                py = mm.tile([128, QB], F32, tag="mm", name=f"y{qb}_{ot}")
                for it in range(4):
                    nc.tensor.matmul(py, wo_sb[it][:, ot * 128:(ot + 1) * 128],
                                     o_sb[it][qb],
                                     start=(it == 0), stop=(it == 3))
                ysb = ysb_p.tile([128, QB], BF16, tag="ysb")
                nc.vector.tensor_copy(ysb, py)
                nc.sync.dma_start(out=yt_d[ot * 128:(ot + 1) * 128, q0:q0 + QB],
                                  in_=ysb)

            # ---- front phase: K/V/Q0 projections for token block 0,
            # t-outer and interleaved so each matmul is paced by its own
            # x/wq tile DMA; then the first V transposes; wo sign-quant ----
            kps = mm.tile([128, QB], F32, tag="mm", name="kps0")
            vps = mm.tile([128, QB], F32, tag="mm", name="vps0")
            qps = scp.tile([128, QB], F32, tag="sc", name="qps00")
            for t in range(NDT):
                nc.tensor.matmul(kps, wk_sb[t], x_sb[t][:, 0:QB],
                                 start=(t == 0), stop=(t == NDT - 1))
                nc.tensor.matmul(vps, wv_sb[t], x_sb[t][:, 0:QB],
                                 start=(t == 0), stop=(t == NDT - 1))
                nc.tensor.matmul(qps, wq_sb[t][:, 0:128], x_sb[t][:, 0:QB],
                                 start=(t == 0), stop=(t == NDT - 1))
            nc.vector.tensor_scalar(k_sb[0], kps, sscore_bc, None, ALU.mult)
            nc.vector.tensor_scalar(vf_sb[0], vps, sout_bc, None, ALU.mult)
            nc.vector.tensor_copy(q_sb[0][0], qps)
            for t in range(4):
                emit_vtok(t)
            for t in range(QF // 128):
                sign_inplace(wo_sb[t])

            # ---- PE filler queue ----
            # Units are (deadline_slot, cost_ns, emit_fn); deadline_slot is
            # the linear slot index (qb*4+ft) at whose START the unit must
            # have been emitted (None = no deadline).  Dripping is paced by
            # a per-iteration credit so filler spreads into the ACT-bound
            # deep query blocks instead of draining eagerly.
            filler = []
            pending_norms = []  # normalizes of the previous slot: emitted at
            # the next slot's first iterations, before its PV matmuls need
            # the po buffers back (PE-queue order, else deadlock)
            credit = [0.0]

            def drip(budget):
                credit[0] += budget
                while filler and credit[0] > 0:
                    _, cost, fn = filler.pop(0)
                    credit[0] -= cost
                    fn()

            def drain_due(slot):
                while any(dl is not None and dl <= slot for dl, _, _ in filler):
                    filler.pop(0)[2]()
                if credit[0] > 0:
                    credit[0] = 0.0

            MM_NS = 215.0

            # ---- attention main loop ----
            for qb in range(NQB):
                # work that becomes available / needed at this query block.
                # K/V + vtok for key blocks 2 and 3 are pushed early (qb0 /
                # qb1) where the PE must stay dense to keep the HAM
                # clock-gate warm; out-proj groups for qb-1 drip during qb,
                # except 4 of qb1's reserved for qb3 (the most ACT-bound).
                if qb == 0:
                    for nxt in (1, 2):
                        filler.append((nxt * 4, 32 * MM_NS,
                                       lambda _q=nxt: kv_block(_q)))
                        filler.append((nxt * 4, 4 * 120.0, lambda _q=nxt: [
                            emit_vtok(4 * _q + i) for i in range(4)]))
                elif qb == 1:
                    filler.append((12, 32 * MM_NS, lambda: kv_block(3)))
                    filler.append((12, 4 * 120.0, lambda: [
                        emit_vtok(12 + i) for i in range(4)]))
                    filler.extend(
                        (None, 4 * MM_NS, (lambda _ot=ot: emit_ygroup(0, _ot)))
                        for ot in range(NDT))
                elif qb == 2:
                    filler.extend(
                        (None, 4 * MM_NS, (lambda _ot=ot: emit_ygroup(1, _ot)))
                        for ot in range(12))
                elif qb == 3:
                    filler.extend(
                        (None, 4 * MM_NS, (lambda _ot=ot: emit_ygroup(1, _ot)))
                        for ot in range(12, NDT))
                    filler.extend(
                        (None, 4 * MM_NS, (lambda _ot=ot: emit_ygroup(2, _ot)))
                        for ot in range(NDT))

                nkt = 4 * (qb + 1)          # causal: key tiles 0..nkt-1
                for ft in range(4):
                    slot = qb * 4 + ft
                    drain_due(slot)
                    # JIT Q projection for the next slot ((0,0) was done in
                    # the front phase).  Appended, not front-inserted: the
                    # queue must stay FIFO so psum chains never interleave.
                    if ft < 3:
                        filler.append((slot + 1, 16 * MM_NS,
                                       lambda _f=ft + 1, _q=qb: q_block(_f, _q)))
                    elif qb + 1 < NQB:
                        filler.append((slot + 1, 16 * MM_NS,
                                       lambda _q=qb + 1: q_block(0, _q)))

                    # po is allocated lazily at the first PV emission
                    # (kt=2): the deferred normalizes of the previous slot
                    # read the po buffers at kt<2, and the pool's WAR
                    # tracking only sees reads emitted before the next
                    # allocation of the tag.  Allocating early loses that
                    # edge and the next slot's PV can overwrite the rowsum
                    # row mid-read (nondeterministic corruption).
                    po_ = []

                    def emit_pv(kt, ex, _po=po_, _nkt=nkt, _qb=qb, _ft=ft):
                        if not _po:
                            _po.extend(
                                pop.tile([HD + 1, QB], F32, tag="po", bufs=2,
                                         name=f"po{_qb}_{_ft}_{p}")
                                for p in range(2))
                        for p in range(2):
                            nc.tensor.matmul(_po[p], vtok[kt][:, p, :],
                                             ex[:, p, :],
                                             start=(kt == 0),
                                             stop=(kt == _nkt - 1))

                    pend = []
                    q0 = qb * QB
                    for kt in range(nkt):
                        ps = scp.tile([128, 2, QB], F32, tag="sc", bufs=2,
                                      name=f"sc{qb}_{ft}_{kt}")
                        kb, kc = kt // 4, kt % 4
                        for p in range(2):
                            r0 = p * HD
                            nc.tensor.matmul(
                                ps[:, p, :],
                                k_sb[kb][r0:r0 + HD, kc * KT:(kc + 1) * KT],
                                q_sb[ft][qb][r0:r0 + HD, :],
                                start=True, stop=True)
                        ex = exps_p.tile([128, 2, QB], BF16, tag="ex", bufs=4,
                                         name=f"ex{qb}_{ft}_{kt}")
                        nc.scalar.activation(out=ex[:, :, :], in_=ps[:, :, :],
                                             func=ACT.Exp)
                        if kt >= 4 * qb:  # diagonal tile: causal mask,
                            # written to a separate tile (in-place RMW on a
                            # cross-engine-consumed tile is race-prone)
                            dmi = kt - 4 * qb
                            exm = exps_p.tile([128, 2, QB], BF16, tag="exm",
                                              bufs=4, name=f"exm{qb}_{ft}_{kt}")
                            for p in range(2):
                                nc.gpsimd.affine_select(
                                    out=exm[:, p, :], in_=ex[:, p, :],
                                    compare_op=ALU.is_ge, fill=0.0,
                                    base=-128 * dmi, pattern=[[1, QB]],
                                    channel_multiplier=-1)
                            ex = exm
                        pend.append((kt, ex))
                        if len(pend) > 2:
                            kt_, ex_ = pend.pop(0)
                            emit_pv(kt_, ex_)
                        # ACT-PE deficit per iteration: exp ~1110ns vs
                        # scores-pair + PV-pair ~645ns (slightly over to
                        # keep the PE ahead of the HAM idle monitor)
                        drip(520.0)
                    for kt_, ex_ in pend:
                        emit_pv(kt_, ex_)

                    # normalize: O[:, q] * (1 / rowsum[q]); rowsum is po row
                    # 64.  The rowsum row is copied to SBUF now (DVE), but
                    # the broadcast matmul + reciprocal + multiply are
                    # dripped as a filler unit so the PE does not stall on
                    # the DVE copy at the slot boundary.
                    def normalize(p, _po=po_, _ft=ft, _qb=qb, _slot=slot):
                        rsum = work.tile([HD + 1, QB], F32, tag="rsum")
                        nc.vector.tensor_copy(rsum[HD:HD + 1, :],
                                              _po[p][HD:HD + 1, :])
                        if _slot == NQB * 4 - 1:
                            rbc = mm.tile([HD, QB], F32, tag="mm")
                            nc.tensor.matmul(rbc, ones64[HD:HD + 1, :],
                                             rsum[HD:HD + 1, :],
                                             start=True, stop=True)
                        else:
                            rid = _slot * 2 + p
                            nc.sync.dma_start(out=rs_d[rid:rid + 1, :],
                                              in_=rsum[HD:HD + 1, :])
                            rbc = work.tile([HD, QB], F32, tag="rbc")
                            nc.sync.dma_start(
                                out=rbc,
                                in_=rs_d[rid:rid + 1, :].to_broadcast([HD, QB]))
                        bcr = work.tile([HD, QB], F32, tag="bcr")
                        nc.vector.reciprocal_approx_fast(out=bcr, in_=rbc)
                        ostg = work.tile([HD, QB], BF16, tag="ostg")
                        nc.vector.tensor_tensor(ostg, _po[p][0:HD, :], bcr,
                                                ALU.mult)
                        nc.sync.dma_start(
                            out=o_sb[_ft][_qb][p * HD:(p + 1) * HD, :],
                            in_=ostg)

                    for p in range(2):
                        normalize(p)

            # drain remaining filler and the last block's out-projection
            while filler:
                filler.pop(0)[2]()
            for ot in range(NDT):
                emit_ygroup(NQB - 1, ot)

    nc.compile()
    return nc


def _get_nc():
    global _NC
    if _NC is None:
        _NC = _build()
    return _NC


def run(inputs, trace=False, trace_cores=None):
    global _LAST_RESULTS
    x = np.asarray(inputs["x"], dtype=np.float32)
    wq = np.asarray(inputs["wq"], dtype=np.float32)
    wk = np.asarray(inputs["wk"], dtype=np.float32)
    wv = np.asarray(inputs["wv"], dtype=np.float32)
    wo = np.asarray(inputs["wo"], dtype=np.float32)

    sq = max(np.abs(wq).mean(), EPS)
    sk = max(np.abs(wk).mean(), EPS)
    sv = max(np.abs(wv).mean(), EPS)
    so = max(np.abs(wo).mean(), EPS)
    sc = np.array([[sq * sk / np.sqrt(HD), sv * so]], dtype=np.float32)

    perm_rows = np.concatenate([np.arange(h * HD, (h + 1) * HD) for h in PERM])

    in_maps = []
    for c in range(8):
        b, g = divmod(c, 4)
        wq_g = wq[QF * g:QF * (g + 1), :][perm_rows]        # [512, 2048]
        wk_g = wk[KF * g:KF * (g + 1), :]                   # [128, 2048]
        wv_g = wv[KF * g:KF * (g + 1), :]
        wo_g = wo[:, QF * g:QF * (g + 1)][:, perm_rows]     # [2048, 512]
        bf = ml_dtypes.bfloat16
        in_maps.append({
            "xt": np.ascontiguousarray(x[b].T).astype(bf),
            "wqt": np.ascontiguousarray(wq_g.T).astype(bf),
            "wkt": np.ascontiguousarray(wk_g.T).astype(bf),
            "wvt": np.ascontiguousarray(wv_g.T).astype(bf),
            "wot": np.ascontiguousarray(wo_g.T).astype(bf),
            "sc": sc,
        })

    nc = _get_nc()
    kwargs = {}
    if trace:
        kwargs["trace"] = True
        kwargs["trace_cores"] = trace_cores if trace_cores is not None else [0]
    res = bass_utils.run_bass_kernel_spmd(nc, in_maps, list(range(8)), **kwargs)
    _LAST_RESULTS = res

    y = np.empty((B, S, D), dtype=np.float32)
    for b in range(B):
        acc = np.zeros((D, S), dtype=np.float32)
        for g in range(4):
            acc += res.results[4 * b + g]["yt"].astype(np.float32)
        y[b] = acc.T
    return y


def kernel(**inputs):
    return run(inputs, trace=False)


# revision 53
# speedup vs baseline: 1.1234x; 1.0342x over previous
# BitAttention (ternary-quantized GQA transformer block) on 8 Trainium2 NeuronCores.
#
# Reference computation (see problem):
#   w_q = sign(w) * mean(|w|)            (per weight tensor, global scale)
#   q = x @ w_q(wq).T ; k = x @ w_q(wk).T ; v = x @ w_q(wv).T
#   GQA causal attention (32 q heads, 8 kv heads, head_dim 64)
#   out = attn @ w_q(wo).T
#
# Sharding (8 cores): batch (2) x kv-head-group (4).  Each core computes
# attention for 2 kv heads / 8 q heads of one batch and a partial out-proj
# over its 512 attention-output features; the host sums 4 partials per batch.
#
# Device layout: activations are feature-major ("transposed", [feat, token]).
# Inputs enter pre-transposed/sliced in bf16; sign() is computed on device;
# the quant scales enter as a tiny [1,2] fp32 tensor and are folded into
# k_sb (sq*sk/sqrt(hd)) and vf_sb (sv*so) at projection-cast time.
#
# Schedule: x loads in 512-token column blocks on the Sync DMA queue while
# the weights load in parallel on the GpSimd DMA queue; the K/V/Q0
# projections for token block 0 run t-outer (one matmul per contraction
# tile as it lands) so the PE starts within a few us and the HAM clock-gate
# warms early.  The attention main loop is ACT(exp)-paced in deep query
# blocks, so all remaining projection work (Q blocks JIT, K/V for later key
# blocks, V transposes, out-proj groups, softmax normalizations) is dripped
# into it as PE filler between score/PV matmuls, paced by a per-iteration
# deficit budget so filler carries forward to the deepest (most ACT-bound)
# query blocks.  Scores are computed transposed ([key, query]) with the two
# heads of a pair on different PE row strips (concurrent); PV lags scores
# by two chunks; the rowsum rides as a ones column on V; causal masks are
# applied in-place on the exp tiles by GPSIMD affine_select.

import sys

for _p in ("/opt/trn_rl_repo",):
    if _p not in sys.path:
        sys.path.append(_p)

import numpy as np
import ml_dtypes

import concourse.bass as bass
import concourse.tile as tile
from concourse import bacc, mybir
from concourse import bass_utils
from concourse.masks import make_identity

F32 = mybir.dt.float32
BF16 = mybir.dt.bfloat16
ALU = mybir.AluOpType
ACT = mybir.ActivationFunctionType

D = 2048          # model dim
S = 2048          # sequence length
B = 2             # batch
HD = 64           # head dim
NQH = 8           # q heads per core
NKV = 2           # kv heads per core
QF = NQH * HD     # 512 q features per core
KF = NKV * HD     # 128 kv features per core
QB = 512          # query block (free dim of score matmuls)
KT = 128          # key tile (partition dim of transposed scores)
NKT = S // KT     # 16
NQB = S // QB     # 4
NDT = D // 128    # 16 contraction tiles
EPS = 1e-5

# processing order of local q heads: tile ft holds heads (ft, ft+4) so that
# the head's row block (64*(h//4)) matches its kv head's row block in k_sb.
PERM = [0, 4, 1, 5, 2, 6, 3, 7]

_NC = None
_LAST_RESULTS = None


def _build():
    nc = bacc.Bacc("TRN2", target_bir_lowering=False, debug=False, num_devices=8)

    xt_d = nc.dram_tensor("xt", [D, S], BF16, kind="ExternalInput")
    wqt_d = nc.dram_tensor("wqt", [D, QF], BF16, kind="ExternalInput")
    wkt_d = nc.dram_tensor("wkt", [D, KF], BF16, kind="ExternalInput")
    wvt_d = nc.dram_tensor("wvt", [D, KF], BF16, kind="ExternalInput")
    wot_d = nc.dram_tensor("wot", [QF, D], BF16, kind="ExternalInput")
    sc_d = nc.dram_tensor("sc", [1, 2], F32, kind="ExternalInput")
    yt_d = nc.dram_tensor("yt", [D, S], BF16, kind="ExternalOutput")
    # scratch rows for the rowsum-reciprocal partition broadcast: DMA can
    # broadcast a DRAM row across partitions but not an SBUF row.  Write
    # and broadcast-read go through the same (Sync) DMA queue, whose FIFO
    # guarantees the ordering.
    rs_d = nc.dram_tensor("rs_scratch", [2 * NQB * 4, QB], F32, kind="Internal")

    with tile.TileContext(nc) as tc:
        with (
            tc.tile_pool(name="persist", bufs=1) as pers,
            tc.tile_pool(name="work", bufs=4) as work,
            tc.tile_pool(name="exps_p", bufs=4) as exps_p,
            tc.tile_pool(name="ysb_p", bufs=4) as ysb_p,
            tc.tile_pool(name="mm", bufs=2, space="PSUM") as mm,
            tc.tile_pool(name="scp", bufs=2, space="PSUM") as scp,
            tc.tile_pool(name="pop", bufs=2, space="PSUM") as pop,
        ):
            # ---- constants ----
            sscore_bc = pers.tile([128, 1], F32, tag="sscore")
            sout_bc = pers.tile([128, 1], F32, tag="sout")
            nc.sync.dma_start(out=sscore_bc, in_=sc_d[0:1, 0:1].to_broadcast([128, 1]))
            nc.sync.dma_start(out=sout_bc, in_=sc_d[0:1, 1:2].to_broadcast([128, 1]))
            ident = pers.tile([128, 128], BF16, tag="ident")
            make_identity(nc, ident)
            # ones row at partition 64 for the rowsum-broadcast matmul
            ones64 = pers.tile([HD + 1, HD], F32, tag="ones64")
            nc.gpsimd.memset(ones64, 1.0)

            # ---- PE warm-up ----
            # Back-to-back matmuls on the identity tile into a scratch psum
            # that is never read.  The HAM clock-gate needs ~3.4us of
            # sustained PE activity to lift the PE clock from 1.2GHz to
            # 2.4GHz; the input DMA window would otherwise leave the PE
            # sparse (and cold) for the first ~50us.  The group is left
            # open; the front-phase projection loop drips more of these
            # between DMA-paced matmuls and closes it.
            wup = scp.tile([128, 128], F32, tag="sc", name="wup")
            for i in range(40):
                nc.tensor.matmul(wup, ident, ident,
                                 start=(i == 0), stop=(i == 39))

            # ---- input DMA ----
            # Sync queue: x column blocks (in block order).  GpSimd queue:
            # weights.  The two queues issue and transfer in parallel, so
            # block 0 of x and the q/k/v weights all land within ~10us.
            wk_sb = [pers.tile([128, KF], BF16, tag=f"wk{t}", name=f"wk{t}")
                     for t in range(NDT)]
            wv_sb = [pers.tile([128, KF], BF16, tag=f"wv{t}", name=f"wv{t}")
                     for t in range(NDT)]
            wq_sb = [pers.tile([128, QF], BF16, tag=f"wq{t}", name=f"wq{t}")
                     for t in range(NDT)]
            wo_sb = [pers.tile([128, D], BF16, tag=f"wo{t}", name=f"wo{t}")
                     for t in range(QF // 128)]
            x_sb = [pers.tile([128, S], BF16, tag=f"x{t}", name=f"x{t}")
                    for t in range(NDT)]

            for t in range(NDT):
                nc.gpsimd.dma_start(out=wk_sb[t], in_=wkt_d[t * 128:(t + 1) * 128, :])
                nc.gpsimd.dma_start(out=wv_sb[t], in_=wvt_d[t * 128:(t + 1) * 128, :])
            for t in range(NDT):
                nc.sync.dma_start(out=x_sb[t][:, 0:QB],
                                  in_=xt_d[t * 128:(t + 1) * 128, 0:QB])
                nc.gpsimd.dma_start(out=wq_sb[t], in_=wqt_d[t * 128:(t + 1) * 128, :])
            for qb in (1, 2, 3):
                for t in range(NDT):
                    nc.sync.dma_start(
                        out=x_sb[t][:, qb * QB:(qb + 1) * QB],
                        in_=xt_d[t * 128:(t + 1) * 128, qb * QB:(qb + 1) * QB])
            for t in range(QF // 128):
                nc.gpsimd.dma_start(out=wo_sb[t], in_=wot_d[t * 128:(t + 1) * 128, :])

            def sign_inplace(w):
                # w <- sign(w) in {-1, +1}: (w >= 0)*2 - 1
                nc.vector.tensor_scalar(w, w, 0.0, 2.0, ALU.is_ge, ALU.mult)
                nc.vector.tensor_scalar(w, w, 1.0, None, ALU.subtract)

            for t in range(NDT):
                sign_inplace(wk_sb[t])
                sign_inplace(wv_sb[t])
            for t in range(NDT):
                sign_inplace(wq_sb[t])

            # ---- persistent activation tiles (one tile per 512-token
            # block so JIT writes and reads of different blocks are tracked
            # as distinct tensors) ----
            k_sb = [pers.tile([128, QB], BF16, tag=f"ksb{qb}", name=f"ksb{qb}")
                    for qb in range(NQB)]
            vf_sb = [pers.tile([128, QB], BF16, tag=f"vfsb{qb}",
                               name=f"vfsb{qb}") for qb in range(NQB)]
            q_sb = [[pers.tile([128, QB], BF16, tag=f"qsb{ft}_{qb}",
                               name=f"qsb{ft}_{qb}") for qb in range(NQB)]
                    for ft in range(4)]
            o_sb = [[pers.tile([128, QB], BF16, tag=f"osb{ft}_{qb}",
                               name=f"osb{ft}_{qb}") for qb in range(NQB)]
                    for ft in range(4)]
            vtok = [pers.tile([128, NKV, HD + 1], BF16, tag=f"vtok{t}",
                              name=f"vtok{t}") for t in range(NKT)]

            # ---- projection emitters (all feature-major, [feat, token]) ----
            def kv_block(qb):
                # K and V projections for token block qb, t-outer so each
                # matmul waits only on its own x tile; scales folded in at
                # the psum->sbuf cast.  Atomic: psum chains must not
                # interleave with other mm-pool allocations (FIFO deadlock).
                c0 = qb * QB
                kps = mm.tile([128, QB], F32, tag="mm", name=f"kps{qb}")
                vps = mm.tile([128, QB], F32, tag="mm", name=f"vps{qb}")
                for t in range(NDT):
                    nc.tensor.matmul(kps, wk_sb[t], x_sb[t][:, c0:c0 + QB],
                                     start=(t == 0), stop=(t == NDT - 1))
                    nc.tensor.matmul(vps, wv_sb[t], x_sb[t][:, c0:c0 + QB],
                                     start=(t == 0), stop=(t == NDT - 1))
                nc.vector.tensor_scalar(k_sb[qb], kps, sscore_bc,
                                        None, ALU.mult)
                nc.vector.tensor_scalar(vf_sb[qb], vps, sout_bc,
                                        None, ALU.mult)

            def emit_vtok(t):
                vt = vtok[t]
                pst = mm.tile([128, 128], BF16, tag="mm", name=f"vt{t}")
                nc.tensor.transpose(
                    pst, vf_sb[t // 4][:, (t % 4) * 128:(t % 4 + 1) * 128],
                    ident)
                for kv in range(NKV):
                    nc.vector.tensor_copy(vt[:, kv, 0:HD],
                                          pst[:, kv * HD:(kv + 1) * HD])
                nc.vector.memset(vt[:, :, HD:HD + 1], 1.0)

            def q_block(ft, qb):
                # Q projection for (q-tile ft, token block qb): 16-matmul
                # accumulation + cast (atomic, see kv_block).
                c0 = qb * QB
                ps = mm.tile([128, QB], F32, tag="mm", name=f"qps{ft}_{qb}")
                for t in range(NDT):
                    nc.tensor.matmul(ps, wq_sb[t][:, ft * 128:(ft + 1) * 128],
                                     x_sb[t][:, c0:c0 + QB],
                                     start=(t == 0), stop=(t == NDT - 1))
                nc.vector.tensor_copy(q_sb[ft][qb], ps)

            def emit_ygroup(qb, ot):
                # one partial out-projection psum group for query block qb
                q0 = qb * QB
                py = mm.tile([128, QB], F32, tag="mm", name=f"y{qb}_{ot}")
                for it in range(4):
                    nc.tensor.matmul(py, wo_sb[it][:, ot * 128:(ot + 1) * 128],
                                     o_sb[it][qb],
                                     start=(it == 0), stop=(it == 3))
                ysb = ysb_p.tile([128, QB], BF16, tag="ysb")
                nc.vector.tensor_copy(ysb, py)
                nc.gpsimd.dma_start(out=yt_d[ot * 128:(ot + 1) * 128, q0:q0 + QB],
                                    in_=ysb)

            # ---- front phase: K/V/Q0 projections for token block 0,
            # t-outer and interleaved so each matmul is paced by its own
            # x/wq tile DMA; then the first V transposes; wo sign-quant ----
            kps = mm.tile([128, QB], F32, tag="mm", name="kps0")
            vps = mm.tile([128, QB], F32, tag="mm", name="vps0")
            qps = scp.tile([128, QB], F32, tag="sc", name="qps00")
            for t in range(NDT):
                nc.tensor.matmul(kps, wk_sb[t], x_sb[t][:, 0:QB],
                                 start=(t == 0), stop=(t == NDT - 1))
                nc.tensor.matmul(vps, wv_sb[t], x_sb[t][:, 0:QB],
                                 start=(t == 0), stop=(t == NDT - 1))
                nc.tensor.matmul(qps, wq_sb[t][:, 0:128], x_sb[t][:, 0:QB],
                                 start=(t == 0), stop=(t == NDT - 1))
            nc.vector.tensor_scalar(k_sb[0], kps, sscore_bc, None, ALU.mult)
            nc.vector.tensor_scalar(vf_sb[0], vps, sout_bc, None, ALU.mult)
            nc.vector.tensor_copy(q_sb[0][0], qps)
            for t in range(4):
                emit_vtok(t)
            for t in range(QF // 128):
                sign_inplace(wo_sb[t])

            # ---- PE filler queue ----
            # Units are (deadline_slot, cost_ns, emit_fn); deadline_slot is
            # the linear slot index (qb*4+ft) at whose START the unit must
            # have been emitted (None = no deadline).  Dripping is paced by
            # a per-iteration credit so filler spreads into the ACT-bound
            # deep query blocks instead of draining eagerly.
            filler = []
            pending_norms = []  # normalizes of the previous slot: emitted at
            # the next slot's first iterations, before its PV matmuls need
            # the po buffers back (PE-queue order, else deadlock)
            credit = [0.0]

            def drip(budget):
                credit[0] += budget
                while filler and credit[0] > 0:
                    _, cost, fn = filler.pop(0)
                    credit[0] -= cost
                    fn()

            def drain_due(slot):
                while any(dl is not None and dl <= slot for dl, _, _ in filler):
                    filler.pop(0)[2]()
                if credit[0] > 0:
                    credit[0] = 0.0

            MM_NS = 215.0

            # ---- attention main loop ----
            for qb in range(NQB):
                # work that becomes available / needed at this query block.
                # K/V + vtok for key blocks 2 and 3 are pushed early (qb0 /
                # qb1) where the PE must stay dense to keep the HAM
                # clock-gate warm; out-proj groups for qb-1 drip during qb,
                # except 4 of qb1's reserved for qb3 (the most ACT-bound).
                if qb == 0:
                    for nxt in (1, 2):
                        filler.append((nxt * 4, 32 * MM_NS,
                                       lambda _q=nxt: kv_block(_q)))
                        filler.append((nxt * 4, 4 * 120.0, lambda _q=nxt: [
                            emit_vtok(4 * _q + i) for i in range(4)]))
                elif qb == 1:
                    filler.append((12, 32 * MM_NS, lambda: kv_block(3)))
                    filler.append((12, 4 * 120.0, lambda: [
                        emit_vtok(12 + i) for i in range(4)]))
                    filler.extend(
                        (None, 4 * MM_NS, (lambda _ot=ot: emit_ygroup(0, _ot)))
                        for ot in range(NDT))
                elif qb == 2:
                    filler.extend(
                        (None, 4 * MM_NS, (lambda _ot=ot: emit_ygroup(1, _ot)))
                        for ot in range(12))
                elif qb == 3:
                    filler.extend(
                        (None, 4 * MM_NS, (lambda _ot=ot: emit_ygroup(1, _ot)))
                        for ot in range(12, NDT))
                    filler.extend(
                        (None, 4 * MM_NS, (lambda _ot=ot: emit_ygroup(2, _ot)))
                        for ot in range(NDT))

                nkt = 4 * (qb + 1)          # causal: key tiles 0..nkt-1
                for ft in range(4):
                    slot = qb * 4 + ft
                    drain_due(slot)
                    # JIT Q projection for the next slot ((0,0) was done in
                    # the front phase).  Appended, not front-inserted: the
                    # queue must stay FIFO so psum chains never interleave.
                    if ft < 3:
                        filler.append((slot + 1, 16 * MM_NS,
                                       lambda _f=ft + 1, _q=qb: q_block(_f, _q)))
                    elif qb + 1 < NQB:
                        filler.append((slot + 1, 16 * MM_NS,
                                       lambda _q=qb + 1: q_block(0, _q)))

                    # po is allocated lazily at the first PV emission
                    # (kt=2): the deferred normalizes of the previous slot
                    # read the po buffers at kt<2, and the pool's WAR
                    # tracking only sees reads emitted before the next
                    # allocation of the tag.  Allocating early loses that
                    # edge and the next slot's PV can overwrite the rowsum
                    # row mid-read (nondeterministic corruption).
                    po_ = []

                    def emit_pv(kt, ex, _po=po_, _nkt=nkt, _qb=qb, _ft=ft):
                        if not _po:
                            _po.extend(
                                pop.tile([HD + 1, QB], F32, tag="po", bufs=2,
                                         name=f"po{_qb}_{_ft}_{p}")
                                for p in range(2))
                        for p in range(2):
                            nc.tensor.matmul(_po[p], vtok[kt][:, p, :],
                                             ex[:, p, :],
                                             start=(kt == 0),
                                             stop=(kt == _nkt - 1))

                    pend = []
                    q0 = qb * QB
                    for kt in range(nkt):
                        ps = scp.tile([128, 2, QB], F32, tag="sc", bufs=2,
                                      name=f"sc{qb}_{ft}_{kt}")
                        kb, kc = kt // 4, kt % 4
                        for p in range(2):
                            r0 = p * HD
                            nc.tensor.matmul(
                                ps[:, p, :],
                                k_sb[kb][r0:r0 + HD, kc * KT:(kc + 1) * KT],
                                q_sb[ft][qb][r0:r0 + HD, :],
                                start=True, stop=True)
                        ex = exps_p.tile([128, 2, QB], BF16, tag="ex", bufs=6,
                                         name=f"ex{qb}_{ft}_{kt}")
                        nc.scalar.activation(out=ex[:, :, :], in_=ps[:, :, :],
                                             func=ACT.Exp)
                        if kt >= 4 * qb:  # diagonal tile: causal mask,
                            # written to a separate tile (in-place RMW on a
                            # cross-engine-consumed tile is race-prone)
                            dmi = kt - 4 * qb
                            exm = exps_p.tile([128, 2, QB], BF16, tag="exm",
                                              bufs=6, name=f"exm{qb}_{ft}_{kt}")
                            for p in range(2):
                                nc.gpsimd.affine_select(
                                    out=exm[:, p, :], in_=ex[:, p, :],
                                    compare_op=ALU.is_ge, fill=0.0,
                                    base=-128 * dmi, pattern=[[1, QB]],
                                    channel_multiplier=-1)
                            ex = exm
                        pend.append((kt, ex))
                        # PV lags scores by 4 chunks: the po buffers are
                        # freed by the previous slot's normalize multiply,
                        # which waits on the rowsum broadcast DMA round
                        # trip (~4us); emitting PV later keeps that WAR off
                        # the PE's critical path.
                        if len(pend) > 4:
                            kt_, ex_ = pend.pop(0)
                            emit_pv(kt_, ex_)
                        # ACT-PE deficit per iteration: exp ~1110ns vs
                        # scores-pair + PV-pair ~645ns (slightly over to
                        # keep the PE ahead of the HAM idle monitor)
                        drip(520.0)
                    for kt_, ex_ in pend:
                        emit_pv(kt_, ex_)

                    # normalize: O[:, q] * (1 / rowsum[q]); rowsum is po row
                    # 64.  The rowsum row is copied to SBUF now (DVE), but
                    # the broadcast matmul + reciprocal + multiply are
                    # dripped as a filler unit so the PE does not stall on
                    # the DVE copy at the slot boundary.
                    def normalize(p, _po=po_, _ft=ft, _qb=qb, _slot=slot):
                        rsum = work.tile([HD + 1, QB], F32, tag="rsum")
                        nc.vector.tensor_copy(rsum[HD:HD + 1, :],
                                              _po[p][HD:HD + 1, :])
                        if _slot == NQB * 4 - 1:
                            rbc = mm.tile([HD, QB], F32, tag="mm")
                            nc.tensor.matmul(rbc, ones64[HD:HD + 1, :],
                                             rsum[HD:HD + 1, :],
                                             start=True, stop=True)
                        else:
                            rid = _slot * 2 + p
                            nc.sync.dma_start(out=rs_d[rid:rid + 1, :],
                                              in_=rsum[HD:HD + 1, :])
                            rbc = work.tile([HD, QB], F32, tag="rbc")
                            nc.sync.dma_start(
                                out=rbc,
                                in_=rs_d[rid:rid + 1, :].to_broadcast([HD, QB]))
                        bcr = work.tile([HD, QB], F32, tag="bcr")
                        nc.vector.reciprocal_approx_fast(out=bcr, in_=rbc)
                        ostg = work.tile([HD, QB], BF16, tag="ostg")
                        nc.vector.tensor_tensor(ostg, _po[p][0:HD, :], bcr,
                                                ALU.mult)
                        nc.sync.dma_start(
                            out=o_sb[_ft][_qb][p * HD:(p + 1) * HD, :],
                            in_=ostg)

                    for p in range(2):
                        normalize(p)

            # drain remaining filler and the last block's out-projection
            while filler:
                filler.pop(0)[2]()
            for ot in range(NDT):
                emit_ygroup(NQB - 1, ot)

    nc.compile()
    return nc


def _get_nc():
    global _NC
    if _NC is None:
        _NC = _build()
    return _NC


def run(inputs, trace=False, trace_cores=None):
    global _LAST_RESULTS
    x = np.asarray(inputs["x"], dtype=np.float32)
    wq = np.asarray(inputs["wq"], dtype=np.float32)
    wk = np.asarray(inputs["wk"], dtype=np.float32)
    wv = np.asarray(inputs["wv"], dtype=np.float32)
    wo = np.asarray(inputs["wo"], dtype=np.float32)

    sq = max(np.abs(wq).mean(), EPS)
    sk = max(np.abs(wk).mean(), EPS)
    sv = max(np.abs(wv).mean(), EPS)
    so = max(np.abs(wo).mean(), EPS)
    sc = np.array([[sq * sk / np.sqrt(HD), sv * so]], dtype=np.float32)

    perm_rows = np.concatenate([np.arange(h * HD, (h + 1) * HD) for h in PERM])

    in_maps = []
    for c in range(8):
        b, g = divmod(c, 4)
        wq_g = wq[QF * g:QF * (g + 1), :][perm_rows]        # [512, 2048]
        wk_g = wk[KF * g:KF * (g + 1), :]                   # [128, 2048]
        wv_g = wv[KF * g:KF * (g + 1), :]
        wo_g = wo[:, QF * g:QF * (g + 1)][:, perm_rows]     # [2048, 512]
        bf = ml_dtypes.bfloat16
        in_maps.append({
            "xt": np.ascontiguousarray(x[b].T).astype(bf),
            "wqt": np.ascontiguousarray(wq_g.T).astype(bf),
            "wkt": np.ascontiguousarray(wk_g.T).astype(bf),
            "wvt": np.ascontiguousarray(wv_g.T).astype(bf),
            "wot": np.ascontiguousarray(wo_g.T).astype(bf),
            "sc": sc,
        })

    nc = _get_nc()
    kwargs = {}
    if trace:
        kwargs["trace"] = True
        kwargs["trace_cores"] = trace_cores if trace_cores is not None else [0]
    res = bass_utils.run_bass_kernel_spmd(nc, in_maps, list(range(8)), **kwargs)
    _LAST_RESULTS = res

    y = np.empty((B, S, D), dtype=np.float32)
    for b in range(B):
        acc = np.zeros((D, S), dtype=np.float32)
        for g in range(4):
            acc += res.results[4 * b + g]["yt"].astype(np.float32)
        y[b] = acc.T
    return y


def kernel(**inputs):
    return run(inputs, trace=False)
